# revision 15
# baseline (speedup 1.0000x reference)
"""Trainium2 Bass kernel for nn_F0Collisions: batched Chang-Cooper implicit
Fokker-Planck solve, 16384 x 512, data-parallel over rows across 8 cores.

Method: each row's tridiagonal system depends on the row only through one
scalar lam = Sg*S4/(6*DV*S2^2) (the 3-step beta fixed point collapses to
beta = 1/T_f to ~1e-11 on this grid).  The Thomas-factorization profiles
alpha_j(lam), betac_j(lam), cp_j(lam) are smooth in lam, so the host builds
Chebyshev-coefficient tables (from the v grid + dt only) and the device:
  1. computes S2/S4/Sg per row (fused multiply+reduce),
  2. evaluates lam and the Chebyshev basis per row,
  3. interpolates the three profiles with one bf16 PE matmul per table
     (P and the tables split into 3 bf16 terms each; all 9 cross products
     stacked along the contraction dim, K = 9M),
  4. runs the forward/backward Thomas sweeps as tensor_tensor_scan linear
     recurrences (the backward sweep streamed in reverse).

Engine balance per 128x512 tile: GpSimd does the f*v2 product and the
betac*f premultiply; ScalarE accumulates S2 and stages PSUM->SBUF copies;
VectorE does the S4/Sg fused reductions and both scans; TensorE does the
basis transpose + 3 matmuls.
"""

import numpy as np
import ml_dtypes

import concourse.bass as bass
import concourse.mybir as mybir
import concourse.tile as tile
from concourse import bacc
from concourse.bass_utils import run_bass_kernel_spmd

NX, NV = 16384, 512
N_CORES = 8
ROWS = NX // N_CORES          # rows per core
NT = ROWS // 128              # 128-row tiles per core
DV = 8.0 / NV
NUEE_COEFF = 2.221e-7
M = 12                        # Chebyshev terms
KSTACK = 9 * M                # stacked contraction dim for split-bf16 matmul

F32 = mybir.dt.float32
BF16 = mybir.dt.bfloat16
ALU = mybir.AluOpType
AFT = mybir.ActivationFunctionType


# ---------------------------------------------------------------- host math

def _host_weights(v):
    """v2 and g weight vectors (float64) s.t. S2 = sum f*v2, Sg = sum f*g."""
    v = v.astype(np.float64)
    v2 = v * v
    we = (0.5 * (v[1:] + v[:-1])) ** 2 * DV / np.sqrt(2.0)   # sqrt_eps * d_eps
    g = np.empty(NV)
    g[0] = 0.5 * we[0]
    g[-1] = 0.5 * we[-1]
    g[1:-1] = 0.5 * (we[:-1] + we[1:])
    return v2, g


def _profiles_for_lam(lam, v, dt):
    """Thomas profiles alpha_j, betac_j, cp_j for a vector of lam (float64)."""
    lam = np.asarray(lam, np.float64)
    v = v.astype(np.float64)
    v2 = v * v
    v_edge = 0.5 * (v[1:] + v[:-1])
    sqrt_eps = v_edge / np.sqrt(2.0)
    D = sqrt_eps[None, :] * lam[:, None]
    C = v_edge[None, :]
    w = C * DV / D
    delta = 1.0 / w - 1.0 / np.expm1(w)
    lo = C * delta - D / DV
    hi = C * (1.0 - delta) + D / DV
    w2 = v_edge ** 2
    w2lo, w2hi = w2 * lo, w2 * hi
    inv = 1.0 / (v2 * DV)
    Mn = lam.shape[0]
    z = np.zeros((Mn, 1))
    diagL = (np.concatenate([w2lo, z], -1) - np.concatenate([z, w2hi], -1)) * inv
    subL = np.concatenate([z, -w2lo], -1) * inv
    supL = np.concatenate([w2hi, z], -1) * inv
    k = float(dt) * NUEE_COEFF
    a = -k * subL
    b = 1.0 - k * diagL
    c = -k * supL
    alpha = np.zeros((Mn, NV))
    betac = np.zeros((Mn, NV))
    cp = np.zeros((Mn, NV))
    cprev = np.zeros(Mn)
    for j in range(NV):
        denom = b[:, j] - a[:, j] * cprev
        cprev = c[:, j] / denom
        cp[:, j] = cprev
        betac[:, j] = 1.0 / denom
        alpha[:, j] = -a[:, j] / denom
    return alpha, betac, cp


def _split3_bf16(X):
    """3-term bf16 split: X ~= h + m + l to ~2^-27 relative."""
    h = X.astype(ml_dtypes.bfloat16)
    r = X - h.astype(np.float32)
    m = r.astype(ml_dtypes.bfloat16)
    l = (r - m.astype(np.float32)).astype(ml_dtypes.bfloat16)
    return np.concatenate([h, m, l], axis=0)   # [3*M, NV]


def _build_tables(f0x, dt, v):
    """Calibrate the lam interval on the actual input and build the split-bf16
    Chebyshev coefficient tables.  Returns (ktab [9M, 3*NV] bf16, mid, half)."""
    f64 = np.asarray(f0x, np.float64)
    v2, g = _host_weights(v)
    v4 = v2 * v2
    S2 = f64 @ v2
    S4 = f64 @ v4
    Sg = f64 @ g
    lam = Sg * S4 / (6.0 * DV * S2 * S2)
    lo, hi = float(lam.min()), float(lam.max())
    span = max(hi - lo, 1e-3 * max(abs(hi), 1e-30))
    lo -= 0.20 * span
    hi += 0.20 * span
    mid = 0.5 * (lo + hi)
    half = 0.5 * (hi - lo)

    kk = np.arange(M)
    xk = np.cos(np.pi * (kk + 0.5) / M)
    lam_nodes = mid + half * xk
    al, bc, cp = _profiles_for_lam(lam_nodes, v, dt)
    T = np.cos(np.outer(np.arange(M), np.pi * (kk + 0.5) / M))
    W = (2.0 / M) * T
    W[0, :] *= 0.5
    tabs = []
    for prof in (al, bc, -cp[:, ::-1]):
        Kc = (W @ prof).astype(np.float32)           # [M, NV]
        Ks = _split3_bf16(Kc)                        # [3M, NV] bf16
        tabs.append(np.tile(Ks, (3, 1)))             # [9M, NV]: (h,m,l)x3
    ktab = np.concatenate(tabs, axis=1)              # [9M, 3*NV]
    return np.ascontiguousarray(ktab), mid, half


# ---------------------------------------------------------------- bass build

def build_program():
    """Build the per-core bass program.  Same program for every core; data
    differs only through the input maps."""
    nc = bacc.Bacc("TRN2", target_bir_lowering=False, debug=False)

    fin = nc.dram_tensor("fin", [ROWS, NV], F32, kind="ExternalInput").ap()
    wtb = nc.dram_tensor("wtb", [128, 4 * 4], BF16, kind="ExternalInput").ap()
    ktab = nc.dram_tensor("ktab", [KSTACK, 3 * NV], BF16,
                          kind="ExternalInput").ap()
    identb = nc.dram_tensor("identb", [128, 128], BF16,
                            kind="ExternalInput").ap()
    ident32 = nc.dram_tensor("ident32", [128, 128], F32,
                             kind="ExternalInput").ap()
    scal = nc.dram_tensor("scal", [128, 2], F32, kind="ExternalInput").ap()
    xout = nc.dram_tensor("xout", [ROWS, NV], F32, kind="ExternalOutput").ap()

    fin_t = fin.rearrange("(t p) j -> t p j", p=128)
    xout_t = xout.rearrange("(t p) j -> t p j", p=128)

    NG = 2                      # pipeline groups
    GT = NT // NG               # tiles per group

    with tile.TileContext(nc) as tc:
        with (
            tc.tile_pool(name="const", bufs=1) as cpool,
            tc.tile_pool(name="work", bufs=3) as wpool,
            tc.tile_pool(name="solve", bufs=3) as spool,
            tc.tile_pool(name="psum_tab", bufs=2, space="PSUM") as tpool,
            tc.tile_pool(name="psum_tr", bufs=2, space="PSUM") as trpool,
        )\
        :
            # --- constants (kt/idn/scs DMAs deferred past the first group's
            # loads so the f-tile DMAs launch first; the sync queue issues
            # configs serially at ~650ns each) ---
            wt = cpool.tile([128, 4 * 4], BF16)
            kt = cpool.tile([KSTACK, 3 * NV], BF16)
            idn = cpool.tile([128, 128], BF16)
            scs = cpool.tile([128, 2], F32)
            idn32 = cpool.tile([128, 128], F32)
            nc.sync.dma_start(wt[:], wtb)
            nc.sync.dma_start(idn32[:], ident32)

            # --- resident f and per-row scalars ---
            fall = cpool.tile([128, NT * NV], F32)
            S2a = cpool.tile([128, NT], F32)
            S4a = cpool.tile([128, NT], F32)
            S0a = cpool.tile([128, NT], F32)
            Sga = cpool.tile([128, NT], F32)
            invS2 = cpool.tile([128, NT], F32)
            u = cpool.tile([128, NT], F32)
            w_ = cpool.tile([128, NT], F32)
            lam = cpool.tile([128, NT], F32)
            xi = cpool.tile([128, NT], F32)
            xi2 = cpool.tile([128, NT], F32)
            tmp = cpool.tile([128, NT], F32)
            F5 = cpool.tile([128, NT], F32)
            q1 = cpool.tile([128, NT], F32)
            Pb = cpool.tile([128, NT * M], F32)
            r1 = cpool.tile([128, NT * M], F32)
            Ph_b = cpool.tile([128, NT * M], BF16)
            Pm_b = cpool.tile([128, NT * M], BF16)
            Pl_b = cpool.tile([128, NT * M], BF16)
            Pstack = cpool.tile([128, NT * KSTACK], BF16)
            fhi = cpool.tile([128, NT * NV], BF16)
            flo = cpool.tile([128, NT * NV], BF16)
            fhiT = cpool.tile([128, NT * 4 * 128], BF16)
            floT = cpool.tile([128, NT * 4 * 128], BF16)
            Sa = cpool.tile([128, NT * 4], F32)
            msb = cpool.tile([4, NT * 128], F32)
            fview = fall[:].rearrange("p (t j) -> p t j", j=NV)
            wtv = wt[:].rearrange("p (c m) -> p c m", m=4)
            fhiTv = fhiT[:].rearrange("p (b r) -> p b r", r=128)
            floTv = floT[:].rearrange("p (b r) -> p b r", r=128)
            Sav = Sa[:].rearrange("p (t m) -> p t m", m=4)
            Pall = Pb[:].rearrange("p (t m) -> p t m", m=M)
            Pst = Pstack[:].rearrange("p (t b m) -> p t b m", b=9, m=M)

            C1 = float(DV / np.sqrt(2.0))
            C2 = float(DV * DV / 4.0)
            C3 = float(32.0 * DV / np.sqrt(2.0))
            CONST = float(1.0 / (6.0 * DV))

            for g in range(NG):
                gsl = slice(g * GT, (g + 1) * GT)
                gm = slice(g * GT * M, (g + 1) * GT * M)

                # ---- phase A: load + moments.  Sg is recovered
                # algebraically: g_j = (DV/sqrt2)*(v_j^2 + DV^2/4) exactly
                # for all j except a -32*(DV/sqrt2)*f[511] boundary term, so
                # Sg = C1*(S2 + C2*S0) - C3*f511.
                for t in range(g * GT, (g + 1) * GT):
                    fsl = fall[:, t * NV:(t + 1) * NV]
                    nc.sync.dma_start(fsl, fin_t[t])
                    # split f = f_hi + f_lo (bf16 pair, exact to ~2^-18)
                    nc.vector.tensor_copy(fhi[:, t * NV:(t + 1) * NV], fsl)
                    nc.gpsimd.tensor_tensor(flo[:, t * NV:(t + 1) * NV], fsl,
                                            fhi[:, t * NV:(t + 1) * NV],
                                            ALU.subtract)
                    # S0 = sum f on ScalarE (its own SBUF port - free)
                    s0d = wpool.tile([128, NV], F32, tag="s0d")
                    nc.scalar.activation(s0d[:], fsl, AFT.Copy,
                                         accum_out=S0a[:, t:t + 1])
                # transposed copies of the halves (2-byte DMA transpose):
                # fT[p, b, r] = f[r, b*128+p], chunk b = t_local*4 + nv_chunk
                gnv = slice(g * GT * NV, (g + 1) * GT * NV)
                gtr = slice(g * GT * 4 * 128, (g + 1) * GT * 4 * 128)
                nc.sync.dma_start_transpose(
                    fhiT[:, gtr].rearrange("p (b r) -> p b r", r=128),
                    fhi[:, gnv])
                nc.sync.dma_start_transpose(
                    floT[:, gtr].rearrange("p (b r) -> p b r", r=128),
                    flo[:, gnv])
                # moments on PE: S[m, row] += sum_j W[j, m] * fT[j, row]
                for t in range(g * GT, (g + 1) * GT):
                    mps = tpool.tile([4, 128], F32, tag="mps", bufs=1)
                    nmm = 0
                    for half in (fhiTv, floTv):
                        for c in range(4):
                            nc.tensor.matmul(
                                mps[:], wtv[:, c, :], half[:, t * 4 + c, :],
                                start=(nmm == 0), stop=(nmm == 7))
                            nmm += 1
                    msl = msb[:, t * 128:(t + 1) * 128]
                    nc.scalar.copy(msl, mps[:])
                    ptm = trpool.tile([128, 4], F32, tag="ptm", bufs=1)
                    nc.tensor.transpose(ptm[:], msl, idn32[:4, :4])
                    nc.scalar.copy(Sav[:, t, :], ptm[:])

                if g == 0:
                    nc.sync.dma_start(kt[:], ktab)
                    nc.sync.dma_start(idn[:], identb)
                    nc.sync.dma_start(scs[:], scal)

                # ---- phase B: per-row scalars -> lam -> Chebyshev basis
                nc.vector.tensor_tensor(S2a[:, gsl], Sav[:, gsl, 0],
                                        Sav[:, gsl, 1], ALU.add)
                nc.vector.tensor_tensor(S4a[:, gsl], Sav[:, gsl, 2],
                                        Sav[:, gsl, 3], ALU.add)
                nc.vector.tensor_copy(F5[:, gsl], fview[:, gsl, 511])
                nc.vector.scalar_tensor_tensor(
                    out=q1[:, gsl], in0=S0a[:, gsl], scalar=C2,
                    in1=S2a[:, gsl], op0=ALU.mult, op1=ALU.add)
                nc.vector.tensor_scalar(out=q1[:, gsl], in0=q1[:, gsl],
                                        scalar1=C1, scalar2=None, op0=ALU.mult)
                nc.vector.scalar_tensor_tensor(
                    out=Sga[:, gsl], in0=F5[:, gsl], scalar=-C3,
                    in1=q1[:, gsl], op0=ALU.mult, op1=ALU.add)
                nc.vector.reciprocal(invS2[:, gsl], S2a[:, gsl])
                nc.vector.tensor_tensor(u[:, gsl], Sga[:, gsl], invS2[:, gsl],
                                        ALU.mult)
                nc.vector.tensor_tensor(w_[:, gsl], S4a[:, gsl], invS2[:, gsl],
                                        ALU.mult)
                nc.vector.scalar_tensor_tensor(
                    out=lam[:, gsl], in0=u[:, gsl], scalar=CONST,
                    in1=w_[:, gsl], op0=ALU.mult, op1=ALU.mult)
                # xi = (lam - mid)/half ; scs[:,0] = -mid, scs[:,1] = 1/half
                nc.vector.tensor_scalar(out=xi[:, gsl], in0=lam[:, gsl],
                                        scalar1=scs[:, 0:1],
                                        scalar2=scs[:, 1:2], op0=ALU.add,
                                        op1=ALU.mult)
                nc.vector.tensor_scalar(out=xi2[:, gsl], in0=xi[:, gsl],
                                        scalar1=2.0, scalar2=None,
                                        op0=ALU.mult)
                nc.vector.memset(Pall[:, gsl, 0], 1.0)
                nc.vector.tensor_copy(Pall[:, gsl, 1], xi[:, gsl])
                for m in range(2, M):
                    nc.vector.tensor_tensor(tmp[:, gsl], xi2[:, gsl],
                                            Pall[:, gsl, m - 1], ALU.mult)
                    nc.vector.tensor_tensor(Pall[:, gsl, m], tmp[:, gsl],
                                            Pall[:, gsl, m - 2], ALU.subtract)
                # split P into 3 bf16 terms h/m/l (mixed-dtype subtracts)
                nc.scalar.copy(Ph_b[:, gm], Pb[:, gm])
                nc.vector.tensor_tensor(r1[:, gm], Pb[:, gm], Ph_b[:, gm],
                                        ALU.subtract)
                nc.scalar.copy(Pm_b[:, gm], r1[:, gm])
                nc.vector.tensor_tensor(r1[:, gm], r1[:, gm], Pm_b[:, gm],
                                        ALU.subtract)
                nc.scalar.copy(Pl_b[:, gm], r1[:, gm])
                # stack 9 blocks tile-major: [h,h,h,m,m,m,l,l,l] per tile
                for b, srcb in enumerate([Ph_b, Ph_b, Ph_b, Pm_b, Pm_b, Pm_b,
                                          Pl_b, Pl_b, Pl_b]):
                    sv = srcb[:].rearrange("p (t m) -> p t m", m=M)
                    nc.scalar.copy(Pst[:, gsl, b, :], sv[:, gsl, :])

                # ---- phase C: tables + solve per tile
                for t in range(g * GT, (g + 1) * GT):
                    fsl = fall[:, t * NV:(t + 1) * NV]
                    ptp = trpool.tile([KSTACK, 128], BF16, tag="ptp", bufs=1)
                    nc.tensor.transpose(
                        ptp[:], Pstack[:, t * KSTACK:(t + 1) * KSTACK], idn[:])
                    lhsT = wpool.tile([KSTACK, 128], BF16, tag="lhsT")
                    nc.scalar.copy(lhsT[:], ptp[:])
                    o_al = tpool.tile([128, NV], F32, tag="o_al", bufs=1)
                    o_bc = tpool.tile([128, NV], F32, tag="o_bc")
                    o_cp = tpool.tile([128, NV], F32, tag="o_cp")
                    nc.tensor.matmul(o_al[:], lhsT[:], kt[:, 0 * NV:1 * NV],
                                     start=True, stop=True)
                    nc.tensor.matmul(o_bc[:], lhsT[:], kt[:, 1 * NV:2 * NV],
                                     start=True, stop=True)
                    nc.tensor.matmul(o_cp[:], lhsT[:], kt[:, 2 * NV:3 * NV],
                                     start=True, stop=True)
                    bc_sb = spool.tile([128, NV], F32, tag="bc_sb")
                    nc.scalar.copy(bc_sb[:], o_bc[:])
                    gt_ = spool.tile([128, NV], F32, tag="gt")
                    nc.gpsimd.tensor_tensor(gt_[:], bc_sb[:], fsl, ALU.mult)
                    dp = spool.tile([128, NV], F32, tag="dp")
                    nc.vector.tensor_tensor_scan(
                        out=dp[:], data0=o_al[:], data1=gt_[:], initial=0.0,
                        op0=ALU.mult, op1=ALU.add)
                    xt = spool.tile([128, NV], F32, tag="xt")
                    nc.vector.tensor_tensor_scan(
                        out=xt[:, ::-1], data0=o_cp[:], data1=dp[:, ::-1],
                        initial=0.0, op0=ALU.mult, op1=ALU.add)
                    nc.sync.dma_start(xout_t[t], xt[:])

    nc.compile()
    return nc


_PROGRAM_CACHE = {}


def _get_program():
    key = "prog"
    if key not in _PROGRAM_CACHE:
        _PROGRAM_CACHE[key] = build_program()
    return _PROGRAM_CACHE[key]


def make_in_maps(f0x, dt, v):
    """Host-side preprocessing: shard f0x, build constant tables."""
    f0x = np.ascontiguousarray(np.asarray(f0x, np.float32))
    v = np.asarray(v, np.float32)
    ktab, mid, half = _build_tables(f0x, float(dt), v)
    v2, g = _host_weights(v)
    v4 = v2 * v2
    # W[j, m] per nv-chunk: columns [bf16(v2), resid, bf16(v4), resid]
    w2h = v2.astype(np.float32).astype(ml_dtypes.bfloat16)
    w2l = (v2.astype(np.float32) - w2h.astype(np.float32)).astype(ml_dtypes.bfloat16)
    w4h = v4.astype(np.float32).astype(ml_dtypes.bfloat16)
    w4l = (v4.astype(np.float32) - w4h.astype(np.float32)).astype(ml_dtypes.bfloat16)
    wtb = np.zeros((128, 4, 4), ml_dtypes.bfloat16)
    for c in range(4):
        sl = slice(c * 128, (c + 1) * 128)
        wtb[:, c, 0] = w2h[sl]
        wtb[:, c, 1] = w2l[sl]
        wtb[:, c, 2] = w4h[sl]
        wtb[:, c, 3] = w4l[sl]
    wtb = wtb.reshape(128, 16)
    identb = np.eye(128, dtype=ml_dtypes.bfloat16)
    ident32 = np.eye(128, dtype=np.float32)
    scal = np.zeros((128, 2), np.float32)
    scal[:, 0] = -mid
    scal[:, 1] = 1.0 / half
    in_maps = []
    for c in range(N_CORES):
        shard = f0x[c * ROWS:(c + 1) * ROWS]
        in_maps.append({
            "fin": np.ascontiguousarray(shard),
            "wtb": wtb, "ktab": ktab, "identb": identb,
            "ident32": ident32, "scal": scal,
        })
    return in_maps


def kernel(nu, f0x, dt, v):
    import os
    nc = _get_program()
    in_maps = make_in_maps(f0x, dt, v)
    trace = bool(os.environ.get("KERNEL_TRACE"))
    res = run_bass_kernel_spmd(nc, in_maps, core_ids=list(range(N_CORES)),
                               trace=trace)
    if trace:
        kernel.last_results = res
    out = np.concatenate([r["xout"] for r in res.results], axis=0)
    return out.astype(np.float32)


# revision 17
# speedup vs baseline: 1.3816x; 1.3816x over previous
"""Trainium2 Bass kernel for nn_F0Collisions: batched Chang-Cooper implicit
Fokker-Planck solve, 16384 x 512, data-parallel over rows across 8 cores.

Method: each row's tridiagonal system depends on the row only through one
scalar lam = Sg*S4/(6*DV*S2^2) (the 3-step beta fixed point collapses to
beta = 1/T_f to ~1e-11 on this grid).  The Thomas-factorization profiles
alpha_j(lam), betac_j(lam), cp_j(lam) are smooth in lam, so the host builds
Chebyshev-coefficient tables (from the v grid + dt only) and the device:
  1. computes S2/S4/Sg per row (fused multiply+reduce),
  2. evaluates lam and the Chebyshev basis per row,
  3. interpolates the three profiles with one bf16 PE matmul per table
     (P and the tables split into 3 bf16 terms each; all 9 cross products
     stacked along the contraction dim, K = 9M),
  4. runs the forward/backward Thomas sweeps as tensor_tensor_scan linear
     recurrences (the backward sweep streamed in reverse).

Engine balance per 128x512 tile: VectorE does the fused S2/S4
multiply-reduces and both scans (the bottleneck engine); ScalarE
accumulates S0 (for the exact Sg identity) and stages PSUM->SBUF copies;
GpSimd does the betac*f premultiply; TensorE does the basis transpose +
3 stacked split-bf16 matmuls.  Tiles run in two pipelined groups so the
second group's moment phase hides the first group's table/solve ramp-up.
"""

import numpy as np
import ml_dtypes

import concourse.bass as bass
import concourse.mybir as mybir
import concourse.tile as tile
from concourse import bacc
from concourse.bass_utils import run_bass_kernel_spmd

NX, NV = 16384, 512
N_CORES = 8
ROWS = NX // N_CORES          # rows per core
NT = ROWS // 128              # 128-row tiles per core
DV = 8.0 / NV
NUEE_COEFF = 2.221e-7
M = 12                        # Chebyshev terms
KSTACK = 9 * M                # stacked contraction dim for split-bf16 matmul

F32 = mybir.dt.float32
BF16 = mybir.dt.bfloat16
ALU = mybir.AluOpType
AFT = mybir.ActivationFunctionType


# ---------------------------------------------------------------- host math

def _host_weights(v):
    """v2 and g weight vectors (float64) s.t. S2 = sum f*v2, Sg = sum f*g."""
    v = v.astype(np.float64)
    v2 = v * v
    we = (0.5 * (v[1:] + v[:-1])) ** 2 * DV / np.sqrt(2.0)   # sqrt_eps * d_eps
    g = np.empty(NV)
    g[0] = 0.5 * we[0]
    g[-1] = 0.5 * we[-1]
    g[1:-1] = 0.5 * (we[:-1] + we[1:])
    return v2, g


def _profiles_for_lam(lam, v, dt):
    """Thomas profiles alpha_j, betac_j, cp_j for a vector of lam (float64)."""
    lam = np.asarray(lam, np.float64)
    v = v.astype(np.float64)
    v2 = v * v
    v_edge = 0.5 * (v[1:] + v[:-1])
    sqrt_eps = v_edge / np.sqrt(2.0)
    D = sqrt_eps[None, :] * lam[:, None]
    C = v_edge[None, :]
    w = C * DV / D
    delta = 1.0 / w - 1.0 / np.expm1(w)
    lo = C * delta - D / DV
    hi = C * (1.0 - delta) + D / DV
    w2 = v_edge ** 2
    w2lo, w2hi = w2 * lo, w2 * hi
    inv = 1.0 / (v2 * DV)
    Mn = lam.shape[0]
    z = np.zeros((Mn, 1))
    diagL = (np.concatenate([w2lo, z], -1) - np.concatenate([z, w2hi], -1)) * inv
    subL = np.concatenate([z, -w2lo], -1) * inv
    supL = np.concatenate([w2hi, z], -1) * inv
    k = float(dt) * NUEE_COEFF
    a = -k * subL
    b = 1.0 - k * diagL
    c = -k * supL
    alpha = np.zeros((Mn, NV))
    betac = np.zeros((Mn, NV))
    cp = np.zeros((Mn, NV))
    cprev = np.zeros(Mn)
    for j in range(NV):
        denom = b[:, j] - a[:, j] * cprev
        cprev = c[:, j] / denom
        cp[:, j] = cprev
        betac[:, j] = 1.0 / denom
        alpha[:, j] = -a[:, j] / denom
    return alpha, betac, cp


def _split3_bf16(X):
    """3-term bf16 split: X ~= h + m + l to ~2^-27 relative."""
    h = X.astype(ml_dtypes.bfloat16)
    r = X - h.astype(np.float32)
    m = r.astype(ml_dtypes.bfloat16)
    l = (r - m.astype(np.float32)).astype(ml_dtypes.bfloat16)
    return np.concatenate([h, m, l], axis=0)   # [3*M, NV]


def _build_tables(f0x, dt, v):
    """Calibrate the lam interval on the actual input and build the split-bf16
    Chebyshev coefficient tables.  Returns (ktab [9M, 3*NV] bf16, mid, half)."""
    f64 = np.asarray(f0x, np.float64)
    v2, g = _host_weights(v)
    v4 = v2 * v2
    S2 = f64 @ v2
    S4 = f64 @ v4
    Sg = f64 @ g
    lam = Sg * S4 / (6.0 * DV * S2 * S2)
    lo, hi = float(lam.min()), float(lam.max())
    span = max(hi - lo, 1e-3 * max(abs(hi), 1e-30))
    lo -= 0.20 * span
    hi += 0.20 * span
    mid = 0.5 * (lo + hi)
    half = 0.5 * (hi - lo)

    kk = np.arange(M)
    xk = np.cos(np.pi * (kk + 0.5) / M)
    lam_nodes = mid + half * xk
    al, bc, cp = _profiles_for_lam(lam_nodes, v, dt)
    T = np.cos(np.outer(np.arange(M), np.pi * (kk + 0.5) / M))
    W = (2.0 / M) * T
    W[0, :] *= 0.5
    tabs = []
    for prof in (al, bc, -cp[:, ::-1]):
        Kc = (W @ prof).astype(np.float32)           # [M, NV]
        Ks = _split3_bf16(Kc)                        # [3M, NV] bf16
        tabs.append(np.tile(Ks, (3, 1)))             # [9M, NV]: (h,m,l)x3
    ktab = np.concatenate(tabs, axis=1)              # [9M, 3*NV]
    return np.ascontiguousarray(ktab), mid, half


# ---------------------------------------------------------------- bass build

def build_program():
    """Build the per-core bass program.  Same program for every core; data
    differs only through the input maps."""
    nc = bacc.Bacc("TRN2", target_bir_lowering=False, debug=False)

    fin = nc.dram_tensor("fin", [ROWS, NV], F32, kind="ExternalInput").ap()
    v2b = nc.dram_tensor("v2b", [128, NV], F32, kind="ExternalInput").ap()
    v4b = nc.dram_tensor("v4b", [128, NV], F32, kind="ExternalInput").ap()
    ktab = nc.dram_tensor("ktab", [KSTACK, 3 * NV], BF16,
                          kind="ExternalInput").ap()
    identb = nc.dram_tensor("identb", [128, 128], BF16,
                            kind="ExternalInput").ap()
    scal = nc.dram_tensor("scal", [128, 2], F32, kind="ExternalInput").ap()
    xout = nc.dram_tensor("xout", [ROWS, NV], F32, kind="ExternalOutput").ap()

    fin_t = fin.rearrange("(t p) j -> t p j", p=128)
    xout_t = xout.rearrange("(t p) j -> t p j", p=128)

    NG = 2                      # pipeline groups
    GT = NT // NG               # tiles per group

    with tile.TileContext(nc) as tc:
        with (
            tc.tile_pool(name="const", bufs=1) as cpool,
            tc.tile_pool(name="work", bufs=3) as wpool,
            tc.tile_pool(name="solve", bufs=3) as spool,
            tc.tile_pool(name="psum_tab", bufs=2, space="PSUM") as tpool,
            tc.tile_pool(name="psum_tr", bufs=2, space="PSUM") as trpool,
        )\
        :
            # --- constants (kt/idn/scs DMAs deferred past the first group's
            # loads so the f-tile DMAs launch first; the sync queue issues
            # configs serially at ~650ns each) ---
            v2s = cpool.tile([128, NV], F32)
            v4s = cpool.tile([128, NV], F32)
            kt = cpool.tile([KSTACK, 3 * NV], BF16)
            idn = cpool.tile([128, 128], BF16)
            scs = cpool.tile([128, 2], F32)
            nc.sync.dma_start(v2s[:], v2b)
            nc.sync.dma_start(v4s[:], v4b)

            # --- resident f and per-row scalars ---
            fall = cpool.tile([128, NT * NV], F32)
            S2a = cpool.tile([128, NT], F32)
            S4a = cpool.tile([128, NT], F32)
            S0a = cpool.tile([128, NT], F32)
            Sga = cpool.tile([128, NT], F32)
            invS2 = cpool.tile([128, NT], F32)
            u = cpool.tile([128, NT], F32)
            w_ = cpool.tile([128, NT], F32)
            lam = cpool.tile([128, NT], F32)
            xi = cpool.tile([128, NT], F32)
            xi2 = cpool.tile([128, NT], F32)
            tmp = cpool.tile([128, NT], F32)
            F5 = cpool.tile([128, NT], F32)
            q1 = cpool.tile([128, NT], F32)
            Pb = cpool.tile([128, NT * M], F32)
            r1 = cpool.tile([128, NT * M], F32)
            Ph_b = cpool.tile([128, NT * M], BF16)
            Pm_b = cpool.tile([128, NT * M], BF16)
            Pl_b = cpool.tile([128, NT * M], BF16)
            Pstack = cpool.tile([128, NT * KSTACK], BF16)
            fview = fall[:].rearrange("p (t j) -> p t j", j=NV)
            Pall = Pb[:].rearrange("p (t m) -> p t m", m=M)
            Pst = Pstack[:].rearrange("p (t b m) -> p t b m", b=9, m=M)

            C1 = float(DV / np.sqrt(2.0))
            C2 = float(DV * DV / 4.0)
            C3 = float(32.0 * DV / np.sqrt(2.0))
            CONST = float(1.0 / (6.0 * DV))

            for g in range(NG):
                gsl = slice(g * GT, (g + 1) * GT)
                gm = slice(g * GT * M, (g + 1) * GT * M)

                # ---- phase A: load + moments.  Sg is recovered
                # algebraically: g_j = (DV/sqrt2)*(v_j^2 + DV^2/4) exactly
                # for all j except a -32*(DV/sqrt2)*f[511] boundary term, so
                # Sg = C1*(S2 + C2*S0) - C3*f511.
                for t in range(g * GT, (g + 1) * GT):
                    fsl = fall[:, t * NV:(t + 1) * NV]
                    nc.sync.dma_start(fsl, fin_t[t])
                    m2 = wpool.tile([128, NV], F32, tag="m2")
                    nc.vector.scalar_tensor_tensor(
                        out=m2[:], in0=fsl, scalar=1.0, in1=v2s[:],
                        op0=ALU.mult, op1=ALU.mult,
                        accum_out=S2a[:, t:t + 1])
                    mdump = wpool.tile([128, NV], F32, tag="mdump")
                    nc.vector.scalar_tensor_tensor(
                        out=mdump[:], in0=fsl, scalar=1.0, in1=v4s[:],
                        op0=ALU.mult, op1=ALU.mult,
                        accum_out=S4a[:, t:t + 1])
                    # S0 = sum f on ScalarE (its own SBUF port - free)
                    s0d = wpool.tile([128, NV], F32, tag="s0d")
                    nc.scalar.activation(s0d[:], fsl, AFT.Copy,
                                         accum_out=S0a[:, t:t + 1])

                if g == 0:
                    nc.sync.dma_start(kt[:], ktab)
                    nc.sync.dma_start(idn[:], identb)
                    nc.sync.dma_start(scs[:], scal)

                # ---- phase B: per-row scalars -> lam -> Chebyshev basis
                nc.vector.tensor_copy(F5[:, gsl], fview[:, gsl, 511])
                nc.vector.scalar_tensor_tensor(
                    out=q1[:, gsl], in0=S0a[:, gsl], scalar=C2,
                    in1=S2a[:, gsl], op0=ALU.mult, op1=ALU.add)
                nc.vector.tensor_scalar(out=q1[:, gsl], in0=q1[:, gsl],
                                        scalar1=C1, scalar2=None, op0=ALU.mult)
                nc.vector.scalar_tensor_tensor(
                    out=Sga[:, gsl], in0=F5[:, gsl], scalar=-C3,
                    in1=q1[:, gsl], op0=ALU.mult, op1=ALU.add)
                nc.vector.reciprocal(invS2[:, gsl], S2a[:, gsl])
                nc.vector.tensor_tensor(u[:, gsl], Sga[:, gsl], invS2[:, gsl],
                                        ALU.mult)
                nc.vector.tensor_tensor(w_[:, gsl], S4a[:, gsl], invS2[:, gsl],
                                        ALU.mult)
                nc.vector.scalar_tensor_tensor(
                    out=lam[:, gsl], in0=u[:, gsl], scalar=CONST,
                    in1=w_[:, gsl], op0=ALU.mult, op1=ALU.mult)
                # xi = (lam - mid)/half ; scs[:,0] = -mid, scs[:,1] = 1/half
                nc.vector.tensor_scalar(out=xi[:, gsl], in0=lam[:, gsl],
                                        scalar1=scs[:, 0:1],
                                        scalar2=scs[:, 1:2], op0=ALU.add,
                                        op1=ALU.mult)
                nc.vector.tensor_scalar(out=xi2[:, gsl], in0=xi[:, gsl],
                                        scalar1=2.0, scalar2=None,
                                        op0=ALU.mult)
                nc.vector.memset(Pall[:, gsl, 0], 1.0)
                nc.vector.tensor_copy(Pall[:, gsl, 1], xi[:, gsl])
                for m in range(2, M):
                    nc.vector.tensor_tensor(tmp[:, gsl], xi2[:, gsl],
                                            Pall[:, gsl, m - 1], ALU.mult)
                    nc.vector.tensor_tensor(Pall[:, gsl, m], tmp[:, gsl],
                                            Pall[:, gsl, m - 2], ALU.subtract)
                # split P into 3 bf16 terms h/m/l (mixed-dtype subtracts)
                nc.scalar.copy(Ph_b[:, gm], Pb[:, gm])
                nc.vector.tensor_tensor(r1[:, gm], Pb[:, gm], Ph_b[:, gm],
                                        ALU.subtract)
                nc.scalar.copy(Pm_b[:, gm], r1[:, gm])
                nc.vector.tensor_tensor(r1[:, gm], r1[:, gm], Pm_b[:, gm],
                                        ALU.subtract)
                nc.scalar.copy(Pl_b[:, gm], r1[:, gm])
                # stack 9 blocks tile-major: [h,h,h,m,m,m,l,l,l] per tile
                for b, srcb in enumerate([Ph_b, Ph_b, Ph_b, Pm_b, Pm_b, Pm_b,
                                          Pl_b, Pl_b, Pl_b]):
                    sv = srcb[:].rearrange("p (t m) -> p t m", m=M)
                    nc.scalar.copy(Pst[:, gsl, b, :], sv[:, gsl, :])

                # ---- phase C: tables + solve per tile
                for t in range(g * GT, (g + 1) * GT):
                    fsl = fall[:, t * NV:(t + 1) * NV]
                    ptp = trpool.tile([KSTACK, 128], BF16, tag="ptp")
                    nc.tensor.transpose(
                        ptp[:], Pstack[:, t * KSTACK:(t + 1) * KSTACK], idn[:])
                    lhsT = wpool.tile([KSTACK, 128], BF16, tag="lhsT")
                    nc.scalar.copy(lhsT[:], ptp[:])
                    o_al = tpool.tile([128, NV], F32, tag="o_al")
                    o_bc = tpool.tile([128, NV], F32, tag="o_bc")
                    o_cp = tpool.tile([128, NV], F32, tag="o_cp")
                    nc.tensor.matmul(o_al[:], lhsT[:], kt[:, 0 * NV:1 * NV],
                                     start=True, stop=True)
                    nc.tensor.matmul(o_bc[:], lhsT[:], kt[:, 1 * NV:2 * NV],
                                     start=True, stop=True)
                    nc.tensor.matmul(o_cp[:], lhsT[:], kt[:, 2 * NV:3 * NV],
                                     start=True, stop=True)
                    bc_sb = spool.tile([128, NV], F32, tag="bc_sb")
                    nc.scalar.copy(bc_sb[:], o_bc[:])
                    gt_ = spool.tile([128, NV], F32, tag="gt")
                    nc.gpsimd.tensor_tensor(gt_[:], bc_sb[:], fsl, ALU.mult)
                    dp = spool.tile([128, NV], F32, tag="dp")
                    nc.vector.tensor_tensor_scan(
                        out=dp[:], data0=o_al[:], data1=gt_[:], initial=0.0,
                        op0=ALU.mult, op1=ALU.add)
                    xt = spool.tile([128, NV], F32, tag="xt")
                    nc.vector.tensor_tensor_scan(
                        out=xt[:, ::-1], data0=o_cp[:], data1=dp[:, ::-1],
                        initial=0.0, op0=ALU.mult, op1=ALU.add)
                    nc.sync.dma_start(xout_t[t], xt[:])

    nc.compile()
    return nc


_PROGRAM_CACHE = {}


def _get_program():
    key = "prog"
    if key not in _PROGRAM_CACHE:
        _PROGRAM_CACHE[key] = build_program()
    return _PROGRAM_CACHE[key]


def make_in_maps(f0x, dt, v):
    """Host-side preprocessing: shard f0x, build constant tables."""
    f0x = np.ascontiguousarray(np.asarray(f0x, np.float32))
    v = np.asarray(v, np.float32)
    ktab, mid, half = _build_tables(f0x, float(dt), v)
    v2, g = _host_weights(v)
    v2b = np.broadcast_to(v2.astype(np.float32), (128, NV)).copy()
    v4b = np.broadcast_to((v2 * v2).astype(np.float32), (128, NV)).copy()
    identb = np.eye(128, dtype=ml_dtypes.bfloat16)
    scal = np.zeros((128, 2), np.float32)
    scal[:, 0] = -mid
    scal[:, 1] = 1.0 / half
    in_maps = []
    for c in range(N_CORES):
        shard = f0x[c * ROWS:(c + 1) * ROWS]
        in_maps.append({
            "fin": np.ascontiguousarray(shard),
            "v2b": v2b, "v4b": v4b, "ktab": ktab, "identb": identb,
            "scal": scal,
        })
    return in_maps


def kernel(nu, f0x, dt, v):
    import os
    nc = _get_program()
    in_maps = make_in_maps(f0x, dt, v)
    trace = bool(os.environ.get("KERNEL_TRACE"))
    res = run_bass_kernel_spmd(nc, in_maps, core_ids=list(range(N_CORES)),
                               trace=trace)
    if trace:
        kernel.last_results = res
    out = np.concatenate([r["xout"] for r in res.results], axis=0)
    return out.astype(np.float32)
